# revision 5
# baseline (speedup 1.0000x reference)
"""DualAttention Trainium2 Bass kernel.

Reference computation (per batch element b, B=1024, L=196, H=1024, A=512):
  spatial  = X @ W_att + b_att            # [L,A],  X = att_features[b] [L,H]
  channel  = X.T @ W_ch + b_ch            # [H,A]
  p_h      = h[b] @ W_h + b_h             # [A]
  logits_s = tanh(spatial + p_h) @ w_alpha (+ b_alpha)   # [L]
  w_sp     = softmax(logits_s)            # [L]   (output)
  out_s    = w_sp @ X                     # [H]   (output)
  logits_c = tanh(channel + p_h) @ w_beta (+ b_beta)     # [H]
  w_ch     = softmax(logits_c)            # [H]
  out_c    = X @ w_ch                     # [L]   (output)

b_alpha/b_beta shift all logits equally -> softmax-invariant -> dropped.

Sharding: batch is split across 8 NeuronCores (128 each). Per core, batch
elements are processed in pairs; all matmuls run in float32r (full PE rate
at moving-dim >= 256). Layout trick: both branches are computed transposed
([A-part, *]-free) so the p_h bias fuses into the ACT tanh as a
per-partition bias and logits come out as free-axis rows via M=1 matmuls,
making softmax cheap. The per-pair X.T tiles (needed for the H-contraction
and the channel weighted sum) are built with TensorE transposes.
"""
import sys
import numpy as np

sys.path.insert(0, '/opt/trn_rl_repo')

import concourse.bass as bass          # noqa: E402
import concourse.bacc as bacc          # noqa: E402
import concourse.mybir as mybir        # noqa: E402
from concourse import tile             # noqa: E402
from concourse.bass_utils import run_bass_kernel_spmd  # noqa: E402

dt = mybir.dt
F32 = dt.float32
R = dt.float32r
AF = mybir.ActivationFunctionType

B, L, H, A = 1024, 196, 1024, 512
NCORES = 8
BLOC = B // NCORES          # 128 batch elements per core
NPAIR = BLOC // 2           # 64 pairs
L0, L1 = 128, L - 128       # 128 + 68
KH = H // 128               # 8 K-chunks over H
MA = A // 128               # 4 M-chunks over A

_CACHE = {}


def build_nc():
    nc = bacc.Bacc('TRN2', target_bir_lowering=False, debug=False,
                   num_devices=NCORES)

    att = nc.declare_dram_parameter("att", [BLOC, L, H], R, isOutput=False)
    h_in = nc.declare_dram_parameter("h", [BLOC, H], R, isOutput=False)
    W_att = nc.declare_dram_parameter("W_att", [H, A], R, isOutput=False)
    W_h = nc.declare_dram_parameter("W_h", [H, A], R, isOutput=False)
    W_ch = nc.declare_dram_parameter("W_ch", [L, A], R, isOutput=False)
    w_alpha = nc.declare_dram_parameter("w_alpha", [A], R, isOutput=False)
    w_beta = nc.declare_dram_parameter("w_beta", [A], R, isOutput=False)
    b_att = nc.declare_dram_parameter("b_att", [A], F32, isOutput=False)
    b_h = nc.declare_dram_parameter("b_h", [A], F32, isOutput=False)
    b_ch = nc.declare_dram_parameter("b_ch", [A], F32, isOutput=False)
    ident = nc.declare_dram_parameter("ident", [128, 128], R, isOutput=False)

    o_ws = nc.declare_dram_parameter("o_ws", [BLOC, H], F32, isOutput=True)
    o_wc = nc.declare_dram_parameter("o_wc", [BLOC, L], F32, isOutput=True)
    o_sp = nc.declare_dram_parameter("o_sp", [BLOC, L], F32, isOutput=True)

    with tile.TileContext(nc) as tc:
        with tc.tile_pool(name="const", bufs=1) as cp, \
             tc.tile_pool(name="px", bufs=4) as px, \
             tc.tile_pool(name="pxt", bufs=10) as pxt, \
             tc.tile_pool(name="pts", bufs=8) as pts, \
             tc.tile_pool(name="ptc", bufs=8) as ptc, \
             tc.tile_pool(name="prow", bufs=8) as prow, \
             tc.tile_pool(name="pcol", bufs=12) as pcol, \
             tc.tile_pool(name="psc", bufs=8) as psc, \
             tc.tile_pool(name="ps_xt", bufs=2, space="PSUM") as ps_xt, \
             tc.tile_pool(name="ps_sc", bufs=3, space="PSUM") as ps_sc, \
             tc.tile_pool(name="ps_sm", bufs=3, space="PSUM") as ps_sm:

            # ---------------- setup: constants ----------------
            id_r = cp.tile([128, 128], R, name="id_r")
            nc.sync.dma_start(out=id_r[:], in_=ident[:])

            Wat = []
            for kc in range(KH):
                w = cp.tile([128, A], R, name=f"Wat{kc}")
                nc.sync.dma_start(out=w[:], in_=W_att[kc * 128:(kc + 1) * 128, :])
                Wat.append(w)
            Wch = []
            for kc, (o, sz) in enumerate(((0, L0), (L0, L1))):
                w = cp.tile([sz, A], R, name=f"Wch{kc}", padded_shape=[128, A])
                nc.sync.dma_start(out=w[:], in_=W_ch[o:o + sz, :])
                Wch.append(w)
            Whh = []
            for kc in range(KH):
                w = cp.tile([128, A], R, name=f"Whh{kc}")
                nc.sync.dma_start(out=w[:], in_=W_h[kc * 128:(kc + 1) * 128, :])
                Whh.append(w)
            wal, wbe = [], []
            wa3 = w_alpha.rearrange("(a p o) -> a p o", p=128, o=1)
            wb3 = w_beta.rearrange("(a p o) -> a p o", p=128, o=1)
            for m in range(MA):
                t = cp.tile([128, 1], R, name=f"wal{m}")
                nc.sync.dma_start(out=t[:], in_=wa3[m])
                wal.append(t)
                t2 = cp.tile([128, 1], R, name=f"wbe{m}")
                nc.sync.dma_start(out=t2[:], in_=wb3[m])
                wbe.append(t2)
            ba3 = b_att.rearrange("(a p o) -> a p o", p=128, o=1)
            bh3 = b_h.rearrange("(a p o) -> a p o", p=128, o=1)
            bc3 = b_ch.rearrange("(a p o) -> a p o", p=128, o=1)
            bias_s, bias_c = [], []
            for m in range(MA):
                ta = cp.tile([128, 1], F32, name=f"ba{m}")
                th = cp.tile([128, 1], F32, name=f"bh{m}")
                tcs = cp.tile([128, 1], F32, name=f"bc{m}")
                nc.sync.dma_start(out=ta[:], in_=ba3[m])
                nc.sync.dma_start(out=th[:], in_=bh3[m])
                nc.sync.dma_start(out=tcs[:], in_=bc3[m])
                bs = cp.tile([128, 1], F32, name=f"bs{m}")
                bc = cp.tile([128, 1], F32, name=f"bc2{m}")
                nc.vector.tensor_add(bs[:], ta[:], th[:])
                nc.vector.tensor_add(bc[:], tcs[:], th[:])
                bias_s.append(bs)
                bias_c.append(bc)

            # ---------------- setup: p_h table ----------------
            # PT[m] [128(A-slice), 128(b)] = (h @ W_h).T ; PS/PC add biases
            h_sb = cp.tile([BLOC, H], R, name="h_sb")
            nc.sync.dma_start(out=h_sb[:], in_=h_in[:, :])
            hT = []
            for kc in range(KH):
                p = ps_xt.tile([128, BLOC], R, name="hT_ps", tag="xt")
                nc.tensor.transpose(p[:], h_sb[:, kc * 128:(kc + 1) * 128], id_r[:])
                s = cp.tile([128, BLOC], R, name=f"hT{kc}")
                nc.vector.tensor_copy(s[:], p[:])
                hT.append(s)
            PS, PC = [], []
            for m in range(MA):
                pt = ps_sc.tile([128, BLOC], F32, name="pt_ps", tag="sc")
                for kc in range(KH):
                    nc.tensor.matmul(pt[:], Whh[kc][:, m * 128:(m + 1) * 128],
                                     hT[kc][:], start=(kc == 0), stop=(kc == KH - 1))
                ps_t = cp.tile([128, BLOC], F32, name=f"PS{m}")
                pc_t = cp.tile([128, BLOC], F32, name=f"PC{m}")
                nc.scalar.activation(ps_t[:], pt[:], AF.Identity, bias=bias_s[m][:])
                nc.scalar.activation(pc_t[:], pt[:], AF.Identity, bias=bias_c[m][:])
                PS.append(ps_t)
                PC.append(pc_t)

            # ---------------- main loop over pairs ----------------
            for p in range(NPAIR):
                b0, b1 = 2 * p, 2 * p + 1
                # X tiles for the two batch elements
                X0, X1 = [], []
                for bi, b in enumerate((b0, b1)):
                    x0 = px.tile([L0, H], R, name="x0", tag="x0")
                    x1 = px.tile([L1, H], R, name="x1", tag="x1",
                                 padded_shape=[128, H])
                    nc.sync.dma_start(out=x0[:], in_=att[b, 0:L0, :])
                    nc.sync.dma_start(out=x1[:], in_=att[b, L0:L, :])
                    X0.append(x0)
                    X1.append(x1)

                # XT[hc] [128, 392] = [X(b0).T | X(b1).T] column blocks
                xt = []
                for hc in range(KH):
                    pp = ps_xt.tile([128, 2 * L], R, name="xt_ps", tag="xt")
                    for bi in range(2):
                        c = bi * L
                        nc.tensor.transpose(
                            pp[:, c:c + L0],
                            X0[bi][:, hc * 128:(hc + 1) * 128], id_r[:])
                        nc.tensor.transpose(
                            pp[:, c + L0:c + L],
                            X1[bi][:, hc * 128:(hc + 1) * 128],
                            id_r[0:L1, 0:L1])
                    s = pxt.tile([128, 2 * L], R, name="xt_sb", tag="xt_sb")
                    nc.vector.tensor_copy(s[:], pp[:])
                    xt.append(s)

                # spatial branch: S.T[m] [128, 392], K=H
                t_s = []
                for m in range(MA):
                    pp = ps_sc.tile([128, 2 * L], F32, name="s_ps", tag="sc")
                    for kc in range(KH):
                        nc.tensor.matmul(pp[:], Wat[kc][:, m * 128:(m + 1) * 128],
                                         xt[kc][:], start=(kc == 0),
                                         stop=(kc == KH - 1))
                    ts = pts.tile([128, 2 * L], R, name="ts_sb", tag="ts")
                    for bi, b in enumerate((b0, b1)):
                        nc.scalar.activation(ts[:, bi * L:(bi + 1) * L],
                                             pp[:, bi * L:(bi + 1) * L],
                                             AF.Tanh, bias=PS[m][:, b:b + 1])
                    t_s.append(ts)

                # channel branch: C.T[m] for each b, [128, 1024], K=L
                t_c = {}
                for m in range(MA):
                    for bi in range(2):
                        for nh in range(2):
                            pp = ps_sc.tile([128, 512], F32, name="c_ps", tag="sc")
                            nc.tensor.matmul(
                                pp[:], Wch[0][:, m * 128:(m + 1) * 128],
                                X0[bi][:, nh * 512:(nh + 1) * 512],
                                start=True, stop=False)
                            nc.tensor.matmul(
                                pp[:], Wch[1][:, m * 128:(m + 1) * 128],
                                X1[bi][:, nh * 512:(nh + 1) * 512],
                                start=False, stop=True)
                            tcc = ptc.tile([128, 512], R, name="tc_sb", tag="tc")
                            b = (b0, b1)[bi]
                            nc.scalar.activation(tcc[:], pp[:], AF.Tanh,
                                                 bias=PC[m][:, b:b + 1])
                            t_c[(m, bi, nh)] = tcc

                # logits_s [1, 392]
                ls = ps_sm.tile([1, 2 * L], F32, name="ls_ps", tag="sm")
                for m in range(MA):
                    nc.tensor.matmul(ls[:], wal[m][:], t_s[m][:],
                                     start=(m == 0), stop=(m == MA - 1))
                # logits_c halves per b [1, 512]
                lc = {}
                for bi in range(2):
                    for nh in range(2):
                        pp = ps_sm.tile([1, 512], F32, name="lc_ps", tag="sm")
                        for m in range(MA):
                            nc.tensor.matmul(pp[:], wbe[m][:], t_c[(m, bi, nh)][:],
                                             start=(m == 0), stop=(m == MA - 1))
                        lc[(bi, nh)] = pp

                # softmax pieces (no max subtraction needed: |logits| <= ~20)
                e_s = prow.tile([1, 2 * L], F32, name="e_s", tag="es")
                nc.scalar.activation(e_s[:], ls[:], AF.Exp)
                r_s = []
                for bi in range(2):
                    ssum = psc.tile([1, 1], F32, name="ssum", tag="sm1")
                    nc.vector.reduce_sum(ssum[:], e_s[:, bi * L:(bi + 1) * L],
                                         axis=mybir.AxisListType.X)
                    rr = psc.tile([1, 1], F32, name="rr", tag="sm1")
                    nc.vector.reciprocal(rr[:], ssum[:])
                    r_s.append(rr)
                e_c = {}
                r_c = []
                for bi in range(2):
                    sc0 = psc.tile([1, 1], F32, name="sc0", tag="sm1")
                    sc1 = psc.tile([1, 1], F32, name="sc1", tag="sm1")
                    for nh, acc in ((0, sc0), (1, sc1)):
                        ee = prow.tile([1, 512], F32, name="e_c", tag="ec")
                        nc.scalar.activation(ee[:], lc[(bi, nh)][:], AF.Exp,
                                             accum_out=acc[:])
                        e_c[(bi, nh)] = ee
                    st = psc.tile([1, 1], F32, name="st", tag="sm1")
                    nc.vector.tensor_add(st[:], sc0[:], sc1[:])
                    rr = psc.tile([1, 1], F32, name="rc", tag="sm1")
                    nc.vector.reciprocal(rr[:], st[:])
                    r_c.append(rr)

                # normalized weight rows; w_spatial rows are an output
                wsp = []
                for bi, b in enumerate((b0, b1)):
                    wr = prow.tile([1, L], F32, name="wsp", tag="wsp")
                    nc.vector.tensor_scalar_mul(wr[:], e_s[:, bi * L:(bi + 1) * L],
                                                r_s[bi][:])
                    nc.sync.dma_start(out=o_sp[b:b + 1, :], in_=wr[:])
                    wsp.append(wr)
                wch = {}
                for bi in range(2):
                    for nh in range(2):
                        wr = prow.tile([1, 512], F32, name="wch", tag="ec")
                        nc.vector.tensor_scalar_mul(wr[:], e_c[(bi, nh)][:],
                                                    r_c[bi][:])
                        wch[(bi, nh)] = wr

                # transpose weight rows into columns
                esp_ps = ps_sm.tile([128, 2], F32, name="esp_ps", tag="sm")
                esp68_ps = ps_sm.tile([L1, 2], F32, name="esp68_ps", tag="sm")
                for bi in range(2):
                    nc.tensor.transpose(esp_ps[:, bi:bi + 1],
                                        wsp[bi][:, 0:128],
                                        id_r[0:1, 0:1].bitcast(F32))
                    nc.tensor.transpose(esp68_ps[:, bi:bi + 1],
                                        wsp[bi][:, 128:L],
                                        id_r[0:1, 0:1].bitcast(F32))
                esp = pcol.tile([128, 2], R, name="esp", tag="col")
                esp68 = pcol.tile([L1, 2], R, name="esp68", tag="col",
                                  padded_shape=[128, 2])
                nc.vector.tensor_copy(esp[:], esp_ps[:])
                nc.vector.tensor_copy(esp68[:], esp68_ps[:])
                ecp = []
                for hc in range(KH):
                    pp = ps_sm.tile([128, 2], F32, name="ecp_ps", tag="sm")
                    for bi in range(2):
                        nh, j = divmod(hc, 4)
                        nc.tensor.transpose(
                            pp[:, bi:bi + 1],
                            wch[(bi, nh)][:, j * 128:(j + 1) * 128],
                            id_r[0:1, 0:1].bitcast(F32))
                    s = pcol.tile([128, 2], R, name="ecp", tag="col")
                    nc.vector.tensor_copy(s[:], pp[:])
                    ecp.append(s)

                # weighted_spatial[b] = w_sp @ X  -> [1, 1024] as 2 halves
                for bi, b in enumerate((b0, b1)):
                    for nh in range(2):
                        pp = ps_sm.tile([1, 512], F32, name="ws_ps", tag="sm")
                        nc.tensor.matmul(pp[:],
                                         esp[:, bi:bi + 1],
                                         X0[bi][:, nh * 512:(nh + 1) * 512],
                                         start=True, stop=False)
                        nc.tensor.matmul(pp[:],
                                         esp68[:, bi:bi + 1],
                                         X1[bi][:, nh * 512:(nh + 1) * 512],
                                         start=False, stop=True)
                        oo = prow.tile([1, 512], F32, name="ws_sb", tag="ec")
                        nc.vector.tensor_copy(oo[:], pp[:])
                        nc.sync.dma_start(
                            out=o_ws[b:b + 1, nh * 512:(nh + 1) * 512], in_=oo[:])

                # weighted_channel pair block-diag: [2, 392]
                pp = ps_sm.tile([2, 2 * L], F32, name="wc_ps", tag="sm")
                for hc in range(KH):
                    nc.tensor.matmul(pp[:], ecp[hc][:], xt[hc][:],
                                     start=(hc == 0), stop=(hc == KH - 1))
                oo = prow.tile([2, 2 * L], F32, name="wc_sb", tag="wc_sb")
                nc.vector.tensor_copy(oo[:], pp[:])
                nc.sync.dma_start(out=o_wc[b0:b0 + 1, :], in_=oo[0:1, 0:L])
                nc.sync.dma_start(out=o_wc[b1:b1 + 1, :], in_=oo[1:2, L:2 * L])

    nc.compile()
    return nc


def _get_nc():
    if "nc" not in _CACHE:
        _CACHE["nc"] = build_nc()
    return _CACHE["nc"]


def run_spmd(in_maps, **kw):
    nc = _get_nc()
    return run_bass_kernel_spmd(nc, in_maps, list(range(NCORES)), **kw)


def make_in_maps(att_features, h, W_att, b_att, W_h, b_h, w_alpha, b_alpha,
                 W_ch, b_ch, w_beta, b_beta):
    att_features = np.ascontiguousarray(np.asarray(att_features, dtype=np.float32))
    h = np.ascontiguousarray(np.asarray(h, dtype=np.float32))
    shared = {
        "W_att": np.ascontiguousarray(np.asarray(W_att, np.float32)),
        "W_h": np.ascontiguousarray(np.asarray(W_h, np.float32)),
        "W_ch": np.ascontiguousarray(np.asarray(W_ch, np.float32)),
        "w_alpha": np.ascontiguousarray(np.asarray(w_alpha, np.float32)),
        "w_beta": np.ascontiguousarray(np.asarray(w_beta, np.float32)),
        "b_att": np.ascontiguousarray(np.asarray(b_att, np.float32)),
        "b_h": np.ascontiguousarray(np.asarray(b_h, np.float32)),
        "b_ch": np.ascontiguousarray(np.asarray(b_ch, np.float32)),
        "ident": np.eye(128, dtype=np.float32),
    }
    in_maps = []
    for c in range(NCORES):
        sl = slice(c * BLOC, (c + 1) * BLOC)
        in_maps.append({"att": att_features[sl], "h": h[sl], **shared})
    return in_maps


def kernel(**inputs):
    in_maps = make_in_maps(**inputs)
    res = run_spmd(in_maps)
    ws = np.concatenate([res.results[c]["o_ws"] for c in range(NCORES)], axis=0)
    wc = np.concatenate([res.results[c]["o_wc"] for c in range(NCORES)], axis=0)
    sp = np.concatenate([res.results[c]["o_sp"] for c in range(NCORES)], axis=0)
    return ws, wc, sp


# revision 6
# speedup vs baseline: 257.7949x; 257.7949x over previous
"""DualAttention Trainium2 Bass kernel.

Reference computation (per batch element b, B=1024, L=196, H=1024, A=512):
  spatial  = X @ W_att + b_att            # [L,A],  X = att_features[b] [L,H]
  channel  = X.T @ W_ch + b_ch            # [H,A]
  p_h      = h[b] @ W_h + b_h             # [A]
  logits_s = tanh(spatial + p_h) @ w_alpha (+ b_alpha)   # [L]
  w_sp     = softmax(logits_s)            # [L]   (output)
  out_s    = w_sp @ X                     # [H]   (output)
  logits_c = tanh(channel + p_h) @ w_beta (+ b_beta)     # [H]
  w_ch     = softmax(logits_c)            # [H]
  out_c    = X @ w_ch                     # [L]   (output)

b_alpha/b_beta shift all logits equally -> softmax-invariant -> dropped.

Sharding: batch is split across 8 NeuronCores (128 each). Per core, batch
elements are processed in pairs; all matmuls run in float32r (full PE rate
at moving-dim >= 256). Layout trick: both branches are computed transposed
([A-part, *]-free) so the p_h bias fuses into the ACT tanh as a
per-partition bias and logits come out as free-axis rows via M=1 matmuls,
making softmax cheap. The per-pair X.T tiles (needed for the H-contraction
and the channel weighted sum) are built with TensorE transposes.
"""
import sys
import numpy as np

sys.path.insert(0, '/opt/trn_rl_repo')

import concourse.bass as bass          # noqa: E402
import concourse.bacc as bacc          # noqa: E402
import concourse.mybir as mybir        # noqa: E402
from concourse import tile             # noqa: E402
from concourse.bass_utils import run_bass_kernel_spmd  # noqa: E402

dt = mybir.dt
F32 = dt.float32
R = dt.float32r
AF = mybir.ActivationFunctionType

B, L, H, A = 1024, 196, 1024, 512
NCORES = 8
BLOC = B // NCORES          # 128 batch elements per core
NPAIR = BLOC // 2           # 64 pairs
L0, L1 = 128, L - 128       # 128 + 68
KH = H // 128               # 8 K-chunks over H
MA = A // 128               # 4 M-chunks over A

_CACHE = {}


def build_nc():
    nc = bacc.Bacc('TRN2', target_bir_lowering=False, debug=False,
                   num_devices=NCORES)

    att = nc.declare_dram_parameter("att", [BLOC, L, H], R, isOutput=False)
    h_in = nc.declare_dram_parameter("h", [BLOC, H], R, isOutput=False)
    W_att = nc.declare_dram_parameter("W_att", [H, A], R, isOutput=False)
    W_h = nc.declare_dram_parameter("W_h", [H, A], R, isOutput=False)
    W_ch = nc.declare_dram_parameter("W_ch", [L, A], R, isOutput=False)
    w_alpha = nc.declare_dram_parameter("w_alpha", [A], R, isOutput=False)
    w_beta = nc.declare_dram_parameter("w_beta", [A], R, isOutput=False)
    b_att = nc.declare_dram_parameter("b_att", [A], F32, isOutput=False)
    b_h = nc.declare_dram_parameter("b_h", [A], F32, isOutput=False)
    b_ch = nc.declare_dram_parameter("b_ch", [A], F32, isOutput=False)
    ident = nc.declare_dram_parameter("ident", [128, 128], R, isOutput=False)

    o_ws = nc.declare_dram_parameter("o_ws", [BLOC, H], F32, isOutput=True)
    o_wc = nc.declare_dram_parameter("o_wc", [BLOC, L], F32, isOutput=True)
    o_sp = nc.declare_dram_parameter("o_sp", [BLOC, L], F32, isOutput=True)

    with tile.TileContext(nc) as tc:
        with tc.tile_pool(name="const", bufs=1) as cp, \
             tc.tile_pool(name="px", bufs=6) as px, \
             tc.tile_pool(name="pxt", bufs=12) as pxt, \
             tc.tile_pool(name="pts", bufs=8) as pts, \
             tc.tile_pool(name="ptc", bufs=12) as ptc, \
             tc.tile_pool(name="prow", bufs=8) as prow, \
             tc.tile_pool(name="pcol", bufs=12) as pcol, \
             tc.tile_pool(name="psc", bufs=8) as psc, \
             tc.tile_pool(name="ps_xt", bufs=2, space="PSUM") as ps_xt, \
             tc.tile_pool(name="ps_sc", bufs=3, space="PSUM") as ps_sc, \
             tc.tile_pool(name="ps_sm", bufs=3, space="PSUM") as ps_sm:

            # ---------------- setup: constants ----------------
            id_r = cp.tile([128, 128], R, name="id_r")
            nc.sync.dma_start(out=id_r[:], in_=ident[:])

            Wat = []
            for kc in range(KH):
                w = cp.tile([128, A], R, name=f"Wat{kc}")
                nc.sync.dma_start(out=w[:], in_=W_att[kc * 128:(kc + 1) * 128, :])
                Wat.append(w)
            Wch = []
            for kc, (o, sz) in enumerate(((0, L0), (L0, L1))):
                w = cp.tile([sz, A], R, name=f"Wch{kc}", padded_shape=[128, A])
                nc.sync.dma_start(out=w[:], in_=W_ch[o:o + sz, :])
                Wch.append(w)
            Whh = []
            for kc in range(KH):
                w = cp.tile([128, A], R, name=f"Whh{kc}")
                nc.sync.dma_start(out=w[:], in_=W_h[kc * 128:(kc + 1) * 128, :])
                Whh.append(w)
            wal, wbe = [], []
            wa3 = w_alpha.rearrange("(a p o) -> a p o", p=128, o=1)
            wb3 = w_beta.rearrange("(a p o) -> a p o", p=128, o=1)
            for m in range(MA):
                t = cp.tile([128, 1], R, name=f"wal{m}")
                nc.sync.dma_start(out=t[:], in_=wa3[m])
                wal.append(t)
                t2 = cp.tile([128, 1], R, name=f"wbe{m}")
                nc.sync.dma_start(out=t2[:], in_=wb3[m])
                wbe.append(t2)
            ba3 = b_att.rearrange("(a p o) -> a p o", p=128, o=1)
            bh3 = b_h.rearrange("(a p o) -> a p o", p=128, o=1)
            bc3 = b_ch.rearrange("(a p o) -> a p o", p=128, o=1)
            bias_s, bias_c = [], []
            for m in range(MA):
                ta = cp.tile([128, 1], F32, name=f"ba{m}")
                th = cp.tile([128, 1], F32, name=f"bh{m}")
                tcs = cp.tile([128, 1], F32, name=f"bc{m}")
                nc.sync.dma_start(out=ta[:], in_=ba3[m])
                nc.sync.dma_start(out=th[:], in_=bh3[m])
                nc.sync.dma_start(out=tcs[:], in_=bc3[m])
                bs = cp.tile([128, 1], F32, name=f"bs{m}")
                bc = cp.tile([128, 1], F32, name=f"bc2{m}")
                nc.vector.tensor_add(bs[:], ta[:], th[:])
                nc.vector.tensor_add(bc[:], tcs[:], th[:])
                bias_s.append(bs)
                bias_c.append(bc)

            # ---------------- setup: p_h table ----------------
            # PT[m] [128(A-slice), 128(b)] = (h @ W_h).T ; PS/PC add biases
            h_sb = cp.tile([BLOC, H], R, name="h_sb")
            nc.sync.dma_start(out=h_sb[:], in_=h_in[:, :])
            hT = []
            for kc in range(KH):
                p = ps_xt.tile([128, BLOC], R, name="hT_ps", tag="xt")
                nc.tensor.transpose(p[:], h_sb[:, kc * 128:(kc + 1) * 128], id_r[:])
                s = cp.tile([128, BLOC], R, name=f"hT{kc}")
                nc.vector.tensor_copy(s[:], p[:])
                hT.append(s)
            PS, PC = [], []
            for m in range(MA):
                pt = ps_sc.tile([128, BLOC], F32, name="pt_ps", tag="sc")
                for kc in range(KH):
                    nc.tensor.matmul(pt[:], Whh[kc][:, m * 128:(m + 1) * 128],
                                     hT[kc][:], start=(kc == 0), stop=(kc == KH - 1))
                ps_t = cp.tile([128, BLOC], F32, name=f"PS{m}")
                pc_t = cp.tile([128, BLOC], F32, name=f"PC{m}")
                nc.scalar.activation(ps_t[:], pt[:], AF.Identity, bias=bias_s[m][:])
                nc.scalar.activation(pc_t[:], pt[:], AF.Identity, bias=bias_c[m][:])
                PS.append(ps_t)
                PC.append(pc_t)

            # ---------------- main loop over pairs ----------------
            for p in range(NPAIR):
                b0, b1 = 2 * p, 2 * p + 1
                # X tiles for the two batch elements
                X0, X1 = [], []
                for bi, b in enumerate((b0, b1)):
                    x0 = px.tile([L0, H], R, name="x0", tag="x0")
                    x1 = px.tile([L1, H], R, name="x1", tag="x1",
                                 padded_shape=[128, H])
                    nc.sync.dma_start(out=x0[:], in_=att[b, 0:L0, :])
                    nc.sync.dma_start(out=x1[:], in_=att[b, L0:L, :])
                    X0.append(x0)
                    X1.append(x1)

                # XT[hc] [128, 392] = [X(b0).T | X(b1).T] column blocks
                xt = []
                for hc in range(KH):
                    pp = ps_xt.tile([128, 2 * L], R, name="xt_ps", tag="xt")
                    for bi in range(2):
                        c = bi * L
                        nc.tensor.transpose(
                            pp[:, c:c + L0],
                            X0[bi][:, hc * 128:(hc + 1) * 128], id_r[:])
                        nc.tensor.transpose(
                            pp[:, c + L0:c + L],
                            X1[bi][:, hc * 128:(hc + 1) * 128],
                            id_r[0:L1, 0:L1])
                    s = pxt.tile([128, 2 * L], R, name="xt_sb", tag="xt_sb")
                    nc.vector.tensor_copy(s[:], pp[:])
                    xt.append(s)

                # spatial branch: S.T[m] [128, 392], K=H
                t_s = []
                for m in range(MA):
                    pp = ps_sc.tile([128, 2 * L], F32, name="s_ps", tag="sc")
                    for kc in range(KH):
                        nc.tensor.matmul(pp[:], Wat[kc][:, m * 128:(m + 1) * 128],
                                         xt[kc][:], start=(kc == 0),
                                         stop=(kc == KH - 1))
                    ts = pts.tile([128, 2 * L], R, name="ts_sb", tag="ts")
                    for bi, b in enumerate((b0, b1)):
                        nc.scalar.activation(ts[:, bi * L:(bi + 1) * L],
                                             pp[:, bi * L:(bi + 1) * L],
                                             AF.Tanh, bias=PS[m][:, b:b + 1])
                    t_s.append(ts)

                # channel branch: C.T[m] for each b, [128, 1024], K=L
                t_c = {}
                for m in range(MA):
                    for bi in range(2):
                        for nh in range(2):
                            pp = ps_sc.tile([128, 512], F32, name="c_ps", tag="sc")
                            nc.tensor.matmul(
                                pp[:], Wch[0][:, m * 128:(m + 1) * 128],
                                X0[bi][:, nh * 512:(nh + 1) * 512],
                                start=True, stop=False)
                            nc.tensor.matmul(
                                pp[:], Wch[1][:, m * 128:(m + 1) * 128],
                                X1[bi][:, nh * 512:(nh + 1) * 512],
                                start=False, stop=True)
                            tcc = ptc.tile([128, 512], R, name="tc_sb", tag="tc")
                            b = (b0, b1)[bi]
                            nc.scalar.activation(tcc[:], pp[:], AF.Tanh,
                                                 bias=PC[m][:, b:b + 1])
                            t_c[(m, bi, nh)] = tcc

                # logits_s [1, 392]
                ls = ps_sm.tile([1, 2 * L], F32, name="ls_ps", tag="sm")
                for m in range(MA):
                    nc.tensor.matmul(ls[:], wal[m][:], t_s[m][:],
                                     start=(m == 0), stop=(m == MA - 1))
                # logits_c halves per b [1, 512]
                lc = {}
                for bi in range(2):
                    for nh in range(2):
                        pp = ps_sm.tile([1, 512], F32, name="lc_ps", tag="sm")
                        for m in range(MA):
                            nc.tensor.matmul(pp[:], wbe[m][:], t_c[(m, bi, nh)][:],
                                             start=(m == 0), stop=(m == MA - 1))
                        lc[(bi, nh)] = pp

                # softmax pieces (no max subtraction needed: |logits| <= ~20)
                e_s = prow.tile([1, 2 * L], F32, name="e_s", tag="es")
                nc.scalar.activation(e_s[:], ls[:], AF.Exp)
                r_s = []
                for bi in range(2):
                    ssum = psc.tile([1, 1], F32, name="ssum", tag="sm1")
                    nc.vector.reduce_sum(ssum[:], e_s[:, bi * L:(bi + 1) * L],
                                         axis=mybir.AxisListType.X)
                    rr = psc.tile([1, 1], F32, name="rr", tag="sm1")
                    nc.vector.reciprocal(rr[:], ssum[:])
                    r_s.append(rr)
                e_c = {}
                r_c = []
                for bi in range(2):
                    sc0 = psc.tile([1, 1], F32, name="sc0", tag="sm1")
                    sc1 = psc.tile([1, 1], F32, name="sc1", tag="sm1")
                    for nh, acc in ((0, sc0), (1, sc1)):
                        ee = prow.tile([1, 512], F32, name="e_c", tag="ec")
                        nc.scalar.activation(ee[:], lc[(bi, nh)][:], AF.Exp,
                                             accum_out=acc[:])
                        e_c[(bi, nh)] = ee
                    st = psc.tile([1, 1], F32, name="st", tag="sm1")
                    nc.vector.tensor_add(st[:], sc0[:], sc1[:])
                    rr = psc.tile([1, 1], F32, name="rc", tag="sm1")
                    nc.vector.reciprocal(rr[:], st[:])
                    r_c.append(rr)

                # normalized weight rows; w_spatial rows are an output
                wsp = []
                for bi, b in enumerate((b0, b1)):
                    wr = prow.tile([1, L], F32, name="wsp", tag="wsp")
                    nc.vector.tensor_scalar_mul(wr[:], e_s[:, bi * L:(bi + 1) * L],
                                                r_s[bi][:])
                    nc.sync.dma_start(out=o_sp[b:b + 1, :], in_=wr[:])
                    wsp.append(wr)
                wch = {}
                for bi in range(2):
                    for nh in range(2):
                        wr = prow.tile([1, 512], F32, name="wch", tag="ec")
                        nc.vector.tensor_scalar_mul(wr[:], e_c[(bi, nh)][:],
                                                    r_c[bi][:])
                        wch[(bi, nh)] = wr

                # transpose weight rows into columns, all into one psum
                # tile: cols 0:2 = w_sp[0:128], 2:4 = w_sp[128:196],
                # 4+2*hc+bi = w_ch chunk hc
                cols_ps = ps_sm.tile([128, 4 + 2 * KH], F32, name="cols_ps",
                                     tag="sm")
                idf = id_r[0:1, 0:1].bitcast(F32)
                for bi in range(2):
                    nc.tensor.transpose(cols_ps[:, bi:bi + 1],
                                        wsp[bi][:, 0:128], idf)
                    nc.tensor.transpose(cols_ps[0:L1, 2 + bi:3 + bi],
                                        wsp[bi][:, 128:L], idf)
                    for hc in range(KH):
                        nh, j = divmod(hc, 4)
                        nc.tensor.transpose(
                            cols_ps[:, 4 + 2 * hc + bi:5 + 2 * hc + bi],
                            wch[(bi, nh)][:, j * 128:(j + 1) * 128], idf)
                cols = pcol.tile([128, 4 + 2 * KH], R, name="cols", tag="col")
                nc.vector.tensor_copy(cols[:], cols_ps[:])
                esp = cols[:, 0:2]
                esp68 = cols[0:L1, 2:4]
                ecp = [cols[:, 4 + 2 * hc:6 + 2 * hc] for hc in range(KH)]

                # weighted_spatial[b] = w_sp @ X  -> [1, 1024] as 2 halves
                for bi, b in enumerate((b0, b1)):
                    for nh in range(2):
                        pp = ps_sm.tile([1, 512], F32, name="ws_ps", tag="sm")
                        nc.tensor.matmul(pp[:],
                                         esp[:, bi:bi + 1],
                                         X0[bi][:, nh * 512:(nh + 1) * 512],
                                         start=True, stop=False)
                        nc.tensor.matmul(pp[:],
                                         esp68[:, bi:bi + 1],
                                         X1[bi][:, nh * 512:(nh + 1) * 512],
                                         start=False, stop=True)
                        oo = prow.tile([1, 512], F32, name="ws_sb", tag="ec")
                        nc.vector.tensor_copy(oo[:], pp[:])
                        nc.sync.dma_start(
                            out=o_ws[b:b + 1, nh * 512:(nh + 1) * 512], in_=oo[:])

                # weighted_channel pair block-diag: [2, 392]
                pp = ps_sm.tile([2, 2 * L], F32, name="wc_ps", tag="sm")
                for hc in range(KH):
                    nc.tensor.matmul(pp[:], ecp[hc], xt[hc][:],
                                     start=(hc == 0), stop=(hc == KH - 1))
                oo = prow.tile([2, 2 * L], F32, name="wc_sb", tag="wc_sb")
                nc.vector.tensor_copy(oo[:], pp[:])
                nc.sync.dma_start(out=o_wc[b0:b0 + 1, :], in_=oo[0:1, 0:L])
                nc.sync.dma_start(out=o_wc[b1:b1 + 1, :], in_=oo[1:2, L:2 * L])

    nc.compile()
    return nc


def _get_nc():
    if "nc" not in _CACHE:
        _CACHE["nc"] = build_nc()
    return _CACHE["nc"]


def run_spmd(in_maps, **kw):
    nc = _get_nc()
    return run_bass_kernel_spmd(nc, in_maps, list(range(NCORES)), **kw)


def make_in_maps(att_features, h, W_att, b_att, W_h, b_h, w_alpha, b_alpha,
                 W_ch, b_ch, w_beta, b_beta):
    att_features = np.ascontiguousarray(np.asarray(att_features, dtype=np.float32))
    h = np.ascontiguousarray(np.asarray(h, dtype=np.float32))
    shared = {
        "W_att": np.ascontiguousarray(np.asarray(W_att, np.float32)),
        "W_h": np.ascontiguousarray(np.asarray(W_h, np.float32)),
        "W_ch": np.ascontiguousarray(np.asarray(W_ch, np.float32)),
        "w_alpha": np.ascontiguousarray(np.asarray(w_alpha, np.float32)),
        "w_beta": np.ascontiguousarray(np.asarray(w_beta, np.float32)),
        "b_att": np.ascontiguousarray(np.asarray(b_att, np.float32)),
        "b_h": np.ascontiguousarray(np.asarray(b_h, np.float32)),
        "b_ch": np.ascontiguousarray(np.asarray(b_ch, np.float32)),
        "ident": np.eye(128, dtype=np.float32),
    }
    in_maps = []
    for c in range(NCORES):
        sl = slice(c * BLOC, (c + 1) * BLOC)
        in_maps.append({"att": att_features[sl], "h": h[sl], **shared})
    return in_maps


def kernel(**inputs):
    in_maps = make_in_maps(**inputs)
    res = run_spmd(in_maps)
    ws = np.concatenate([res.results[c]["o_ws"] for c in range(NCORES)], axis=0)
    wc = np.concatenate([res.results[c]["o_wc"] for c in range(NCORES)], axis=0)
    sp = np.concatenate([res.results[c]["o_sp"] for c in range(NCORES)], axis=0)
    return ws, wc, sp
